# revision 19
# baseline (speedup 1.0000x reference)
"""AvgPool2d(64x64, stride 1) with replicate-padding back to (512, 512),
as a distributed Bass kernel on 8 TRN2 NeuronCores.

Input : x (8, 64, 512, 512) float32
Output: (8, 64, 512, 512) float32

Strategy (pure data parallel): one batch element per core. Per core the
pooling is a separable 64-wide box filter; both directions run on the
TensorEngine as matmuls against a banded 0/1-matrix `band` [512, 512]
with band[h, i] = 1/64 iff clamp(i-31, 0, 448) <= h < clamp(...) + 64
(the clamp folds the replicate-padding, the 1/64 folds the averaging).

    V^T = (X^T @ band)        pass 1: vertical box mean, transposed
    O   = (V^T)^T @ band      pass 2: horizontal box mean, natural

Both passes keep the *data* tile in the stationary (lhsT) operand and
the band in the moving operand, which avoids every transpose.

v2 (trace-driven rework of the 216-222 us baseline):
- Tail: the baseline dispatched the last 4-channel group store (1.66MB)
  only after the final matmul -> ~26 us of unoverlapped drain (last
  matmul at 201.7 us, kernel end 228 us). Now channels 60..63 store
  per-channel, split into partition-quarter dma_starts on the sync
  queue (idle at kernel end), dispatched as soon as each channel's two
  PSUM->SBUF copies land.
- Start: the band was one dma_start -> one DMA engine (22.5 GB/s) and
  input c0 was a single dma_start; first matmul at 12.4 us. Now the
  band loads as 4 partition-slices on scalar (idle at t=0) and the
  first channels' loads are partition-split across many engines.
- Input loads ride sync as 2-channel pairs (8KB per-partition
  descriptors, 32 dma_starts instead of 64): halves the 565ns/dispatch
  sync-engine cost and the event-semaphore count.
- Pass-1 PSUM tiles are 2-bank pairs copied once ([128, 2, 456], one
  copy instead of two): ~11% less scalar/vector copy time. Copies stay
  greedily balanced between scalar and vector (the only PSUM-capable
  engines).
- Only output columns [31, 480) (the 449 valid ones) are computed in
  pass 2 / copied / stored; rows keep the quad-aligned [28, 484) cover.
  Store descriptors stay >= 14KB (4-channel groups) for full per-engine
  DMA rate (measured 23.9 GB/s at 14.6KB vs 177ns/4KB reads).
- Deep input prefetch (7 pairs = 14 channels in flight) front-loads the
  read stream while stores are scarce, and keeps the PE p-state hot
  (PE restarts at 1.2 GHz for 3 us after any idle; the trace showed
  30.5 us of throttle).
"""

import numpy as np
import ml_dtypes

C, H, W = 64, 512, 512
P = 128
NKH = H // P  # 4 partition blocks
KERNEL = 64
OUT_VALID = H - KERNEL + 1  # 449
PT = (H - OUT_VALID) // 2  # 31 (left/top pad)
OLO, OHI = 28, 484  # computed output-row range (quad-aligned cover of valid)
NOUT = OHI - OLO  # 456 rows
JLO, JHI = PT, PT + OUT_VALID  # stored output-col range [31, 480)
NJ = JHI - JLO  # 449 cols
NP_OUT = NOUT // 4  # 114 output partitions
G = 4  # channels per grouped store (>=14KB descriptors)
NG_GROUPED = 14  # groups 0..13 cover channels 0..55
C_TAIL = NG_GROUPED * G  # 56: channels >= this store as split 2-ch groups
NPAIR = C // 2  # input channel pairs

# Matmul plan: contraction over standard 128-row blocks k, ONE
# instruction per k (4 LDWEIGHTS per block-pass instead of 7 — the
# per-column 64-row window spans at most 2 k-blocks, and PSUM's
# per-element has_written bit makes "accumulate onto a never-written
# element" a plain write, so a single instruction may mix first-writer
# and accumulating columns): (k, lo, hi, start, stop).
MM_PLAN_BLOCK = [
    (0, 0, 159, True, False),
    (1, 96, 287, False, False),
    (2, 224, 415, False, False),
    (3, 352, 512, False, True),
]
# Pass-1 plan: i-range clipped to the stored rows [OLO, OHI).
MM_PLAN_P1 = [
    (k, max(lo, OLO), min(hi, OHI), start, stop)
    for (k, lo, hi, start, stop) in MM_PLAN_BLOCK
]
# Pass-2 plan: j-range clipped to the valid cols [JLO, JHI).
MM_PLAN_P2 = [
    (k, max(lo, JLO), min(hi, JHI), start, stop)
    for (k, lo, hi, start, stop) in MM_PLAN_BLOCK
]


def make_band() -> np.ndarray:
    i = np.arange(H)
    ic = np.clip(i - PT, 0, OUT_VALID - 1)
    h = np.arange(H)
    band = (h[:, None] >= ic[None, :]) & (h[:, None] < ic[None, :] + KERNEL)
    band = (band.astype(np.float32) / KERNEL).astype(ml_dtypes.bfloat16)
    # pre-block to [p, k, i] = band[128k+p, i] so the DMA reads 4KB chunks
    return np.ascontiguousarray(band.reshape(NKH, P, W).transpose(1, 0, 2))


class CopyBalancer:
    """Greedy build-time assignment of PSUM->SBUF copies to the two
    engines with PSUM ports, weighted by their measured per-copy cost."""

    def __init__(self, nc):
        self.nc = nc
        self.load = {"scalar": 0.0, "vector": 0.0}

    def copy(self, dst, src, fd):
        cost = {"scalar": 250 + fd / 1.2, "vector": (120 + fd) / 0.96}
        eng = min(cost, key=lambda e: self.load[e] + cost[e])
        self.load[eng] += cost[eng]
        if eng == "scalar":
            self.nc.scalar.copy(dst, src)
        else:
            self.nc.vector.tensor_copy(dst, src)


def build_avgpool(tc, x_ap, band_ap, out_ap, out_tail_ap):
    import concourse.mybir as mybir

    nc = tc.nc
    f32 = mybir.dt.float32
    bf16 = mybir.dt.bfloat16
    cb = CopyBalancer(nc)

    with (
        tc.tile_pool(name="const", bufs=1) as const_pool,
        tc.tile_pool(name="xin", bufs=14) as xin_pool,
        tc.tile_pool(name="vt", bufs=3) as vt_pool,
        tc.tile_pool(name="oout", bufs=3) as out_pool,
        tc.tile_pool(name="otail", bufs=4) as tail_pool,
        tc.tile_pool(name="vtps", bufs=2, space="PSUM") as vt_psum,
        tc.tile_pool(name="ops", bufs=2, space="PSUM") as o_psum,
    ):
        # band pre-blocked on host: [p, k, i] = band[128*k + p, i];
        # one dma_start on the scalar engine lands on its first (early,
        # ~5.5 us ring-up) HWDGE queue, parallel to channel 0's load on
        # sync. (Splitting it or using the 2nd scalar ring starts ~4 us
        # later and delays the first matmul — measured.)
        band_t = const_pool.tile([P, NKH, H], bf16, tag="band")
        nc.scalar.dma_start(band_t[:], band_ap)

        # -------- input loads: per channel on sync, 4KB descriptors ----
        xtiles = {}

        def load_chan(c, splits=1):
            t = xin_pool.tile([P, NKH, W], bf16, tag="xb")
            if splits == 1:
                nc.sync.dma_start(t[:], x_ap[c])
            else:
                step = P // splits
                for s in range(splits):
                    sl = slice(step * s, step * (s + 1))
                    nc.sync.dma_start(t[sl], x_ap[c, sl])
            xtiles[c] = t

        # warmup: fill the prefetch window (deep prefetch front-loads
        # the read stream while stores are scarce)
        for c in range(14):
            load_chan(c)

        vtbs = {}

        def pass1(c):
            if c + 14 < C:
                load_chan(c + 14)
            xb = xtiles.pop(c)
            # vtb padded to 512 cols: pass-2 lhsT slices t::4 must be
            # 128-wide or FWL turns off (needs NumWeights==128) and
            # LDWEIGHTS runs 2x slow; pad cols feed only PSUM partitions
            # >= 114, which are never copied out
            vtb = vt_pool.tile([P, NKH, W], bf16, tag="vtb")
            vtbs[c] = vtb
            # 2-bank PSUM pair tiles: one copy per 2 mw-blocks halves
            # the per-copy fixed cost vs single banks
            for mp in range(2):
                vt_ps = vt_psum.tile([P, 2, W], f32, tag="vt")
                for sub in range(2):
                    mw = 2 * mp + sub
                    for k, lo, hi, start, stop in MM_PLAN_P1:
                        nc.tensor.matmul(
                            vt_ps[:, sub, lo - OLO : hi - OLO],
                            xb[:, k, P * mw : P * (mw + 1)],
                            band_t[:, k, lo:hi],
                            start=start,
                            stop=stop,
                        )
                cb.copy(
                    vtb[:, 2 * mp : 2 * mp + 2, :NOUT],
                    vt_ps[:, :, :NOUT],
                    2 * NOUT,
                )

        osbs = {}

        def pass2(c):
            g, ci = divmod(c, G)
            tail = c >= C_TAIL
            if tail:
                tg, tci = divmod(c - C_TAIL, 2)
                if tci == 0:
                    o_tl_new = tail_pool.tile([NP_OUT, 2, 4, NJ], bf16, tag="otl")
                    osbs[("t", tg)] = o_tl_new
                o_sb = osbs[("t", tg)]
                ci = tci
            else:
                if ci == 0:
                    o_sb_new = out_pool.tile([NP_OUT, G, 4, NJ], bf16, tag="osb")
                    osbs[g] = o_sb_new
                o_sb = osbs[g]
            vtb = vtbs.pop(c)
            for hf in range(2):
                # m-slices padded to 512 for PSUM bank alignment; only
                # j in [JLO, JHI) is computed — the trimmed columns are
                # edge replication the host reapplies
                o_ps = o_psum.tile([P, 2, W], f32, tag="o")
                for m in range(2):
                    t = 2 * hf + m
                    for k, lo, hi, start, stop in MM_PLAN_P2:
                        nc.tensor.matmul(
                            o_ps[:, m, lo - JLO : hi - JLO],
                            vtb[:, k, t : W : 4],
                            band_t[:, k, lo:hi],
                            start=start,
                            stop=stop,
                        )
                cb.copy(
                    o_sb[:, ci, 2 * hf : 2 * hf + 2, :],
                    o_ps[:NP_OUT, :, :NJ],
                    2 * NJ,
                )
            if tail and ci == 1:
                # Tail (c >= 56): 2-channel groups, each split into 6
                # partition-slices of 19 descriptors. A DMA engine
                # window is ~38 descriptors regardless of byte size, so
                # a whole group on one dma_start drains over only ~3
                # windows (~8us-24us); 19-desc slices each get their own
                # window (~135KB, ~6.2us) and the final group drains
                # right after its last copy. Queue choice avoids
                # serializing any one engine's ~0.6-0.9us-per-dispatch
                # stream: gpsimd for the first groups, sync once input
                # dispatching is idle, scalar only for the last group
                # (a blocked dma_start in scalar's stream would stall
                # its remaining copies). 7184B descriptors keep the
                # per-engine rate (3.6KB descs crawl).
                o_tl = osbs.pop(("t", tg))
                engs = [
                    [nc.gpsimd] * 6,
                    [nc.sync] * 6,
                    [nc.gpsimd] * 3 + [nc.sync] * 3,
                    [nc.sync] * 3 + [nc.scalar] * 3,
                ][tg]
                for s in range(6):
                    lo, hi = 19 * s, 19 * (s + 1)
                    engs[s].dma_start(out_tail_ap[tg, lo:hi], o_tl[lo:hi])
            elif not tail and ci == G - 1:
                # one >=14KB-per-partition store per 4-channel group,
                # dispatched from the otherwise-idle GpSimd (SWDGE)
                # queue (a HWDGE dispatch would block the sync/scalar
                # instruction stream on this group's copy dependencies,
                # starving input prefetch / later copies)
                nc.gpsimd.dma_start(out_ap[g], osbs.pop(g)[:])

        # software pipeline: PE runs pass1(c+1) while pass2(c) waits on
        # pass-1 copies
        pass1(0)
        for c in range(1, C):
            pass1(c)
            pass2(c - 1)
        pass2(C - 1)


def build_nc():
    import concourse.mybir as mybir
    import concourse.tile as tile
    from concourse import bacc

    # Bacc (not raw Bass): its compile() runs generate_event_semaphores,
    # which splits multi-semaphore waits — walrus codegen allows at most
    # one wait command per DMA instruction.
    nc = bacc.Bacc()
    x = nc.dram_tensor(
        "x", [C, P, NKH, W], mybir.dt.bfloat16, kind="ExternalInput"
    )
    band = nc.dram_tensor(
        "band", [P, NKH, H], mybir.dt.bfloat16, kind="ExternalInput"
    )
    out = nc.dram_tensor(
        "out",
        [NG_GROUPED, NP_OUT, G, 4, NJ],
        mybir.dt.bfloat16,
        kind="ExternalOutput",
    )
    out_tail = nc.dram_tensor(
        "out_tail",
        [(C - C_TAIL) // 2, NP_OUT, 2, 4, NJ],
        mybir.dt.bfloat16,
        kind="ExternalOutput",
    )
    with tile.TileContext(nc) as tc:
        build_avgpool(tc, x.ap(), band.ap(), out.ap(), out_tail.ap())
    nc.compile()
    return nc


def _ensure_axon_ntff_hook():
    """If tracing is requested (BASS_TRACE) under axon, run_bass_kernel_spmd
    imports antenv.axon_hooks, which some agent images lack. Install the
    real hook if possible, else a stub that degrades tracing gracefully."""
    import sys
    import types

    try:
        import antenv.axon_hooks  # noqa: F401

        return
    except Exception:
        pass
    try:
        import antenv
    except Exception:
        return
    mod = types.ModuleType("antenv.axon_hooks")
    mod._hook = None
    mod.set_axon_ntff_profile_hook = lambda h: setattr(mod, "_hook", h)
    mod.get_axon_ntff_profile_hook = lambda: mod._hook
    sys.modules["antenv.axon_hooks"] = mod
    antenv.axon_hooks = mod
    try:
        from trn_agent_boot.trn_boot import _ntff_profile_via_ctypes

        hook = _ntff_profile_via_ctypes("/opt/axon/libaxon_pjrt.so")
        if hook is not None:
            mod.set_axon_ntff_profile_hook(hook)
    except Exception:
        pass


def prep_inputs(x: np.ndarray):
    """Shard, cast, and pre-block the full input for the 8 cores.

    Device x layout: x_dev[c, p, k, w] = x[c, 128k+p, w]
    (4KB contiguous per partition per channel dma).
    """
    x = np.asarray(x, dtype=np.float32)
    assert x.shape == (8, C, H, W)
    xb = x.astype(ml_dtypes.bfloat16)
    band = make_band()
    in_maps = []
    for b in range(8):
        xd = np.ascontiguousarray(
            xb[b].reshape(C, NKH, P, W).transpose(0, 2, 1, 3)
        )
        in_maps.append({"x": xd, "band": band})
    return in_maps


def gather_output(results) -> np.ndarray:
    """Unshard, un-block, upcast, and re-apply the replicate padding.

    Device out layout (grouped, c < 60): out[g, p, ci, t, s] =
      out_full[4g+ci, OLO+4p+t, JLO+s]
    Device out layout (tail, c >= 60): out_tail[tg, p, tci, t, s] with
      c = 60 + 2*tg + tci, same row/col mapping.
    Stored rows [28, 484) include the replicated rows 28..30 / 480..483
    (the band clamp makes them exact copies of rows 31 / 479), so only
    rows outside [28, 484) and cols outside [31, 480) are reapplied.
    """
    full = np.empty((8, C, H, W), dtype=np.float32)
    for b, r in enumerate(results):
        og = np.asarray(r["out"])  # [15, 114, G, 4, 449] bf16
        ot = np.asarray(r["out_tail"])  # [2, 114, 2, 4, 449] bf16
        rows = np.empty((C, NOUT, NJ), dtype=np.float32)
        rows[:C_TAIL] = (
            og.transpose(0, 2, 1, 3, 4).reshape(C_TAIL, NOUT, NJ)
        )
        rows[C_TAIL:] = (
            ot.transpose(0, 2, 1, 3, 4).reshape(C - C_TAIL, NOUT, NJ)
        )
        full[b, :, OLO:OHI, JLO:JHI] = rows
        full[b, :, OLO:OHI, :JLO] = rows[:, :, :1]
        full[b, :, OLO:OHI, JHI:] = rows[:, :, -1:]
        full[b, :, :OLO, :] = full[b, :, OLO : OLO + 1, :]
        full[b, :, OHI:, :] = full[b, :, OHI - 1 : OHI, :]
    return full


def kernel(x) -> np.ndarray:
    _ensure_axon_ntff_hook()
    from concourse.bass_utils import run_bass_kernel_spmd

    nc = build_nc()
    in_maps = prep_inputs(x)
    res = run_bass_kernel_spmd(nc, in_maps, core_ids=list(range(8)))
    return gather_output(res.results)


# revision 22
# speedup vs baseline: 1.3229x; 1.3229x over previous
"""AvgPool2d(64x64, stride 1) with replicate-padding back to (512, 512),
as a distributed Bass kernel on 8 TRN2 NeuronCores.

Input : x (8, 64, 512, 512) float32
Output: (8, 64, 512, 512) float32

Strategy (pure data parallel): one batch element per core. Per core the
pooling is a separable 64-wide box filter; both directions run on the
TensorEngine as matmuls against a banded 0/1-matrix `band` [512, 512]
with band[h, i] = 1/64 iff clamp(i-31, 0, 448) <= h < clamp(...) + 64
(the clamp folds the replicate-padding, the 1/64 folds the averaging).

    V^T = (X^T @ band)        pass 1: vertical box mean, transposed
    O   = (V^T)^T @ band      pass 2: horizontal box mean, natural

Both passes keep the *data* tile in the stationary (lhsT) operand and
the band in the moving operand, which avoids every transpose.

v2 (trace-driven rework of the 216-222 us baseline):
- Tail: the baseline dispatched the last 4-channel group store (1.66MB)
  only after the final matmul -> ~26 us of unoverlapped drain (last
  matmul at 201.7 us, kernel end 228 us). Now channels 60..63 store
  per-channel, split into partition-quarter dma_starts on the sync
  queue (idle at kernel end), dispatched as soon as each channel's two
  PSUM->SBUF copies land.
- Start: the band was one dma_start -> one DMA engine (22.5 GB/s) and
  input c0 was a single dma_start; first matmul at 12.4 us. Now the
  band loads as 4 partition-slices on scalar (idle at t=0) and the
  first channels' loads are partition-split across many engines.
- Input loads ride sync as 2-channel pairs (8KB per-partition
  descriptors, 32 dma_starts instead of 64): halves the 565ns/dispatch
  sync-engine cost and the event-semaphore count.
- Pass-1 PSUM tiles are 2-bank pairs copied once ([128, 2, 456], one
  copy instead of two): ~11% less scalar/vector copy time. Copies stay
  greedily balanced between scalar and vector (the only PSUM-capable
  engines).
- Only output columns [31, 480) (the 449 valid ones) are computed in
  pass 2 / copied / stored; rows keep the quad-aligned [28, 484) cover.
  Store descriptors stay >= 14KB (4-channel groups) for full per-engine
  DMA rate (measured 23.9 GB/s at 14.6KB vs 177ns/4KB reads).
- Deep input prefetch (7 pairs = 14 channels in flight) front-loads the
  read stream while stores are scarce, and keeps the PE p-state hot
  (PE restarts at 1.2 GHz for 3 us after any idle; the trace showed
  30.5 us of throttle).
"""

import numpy as np
import ml_dtypes

C, H, W = 64, 512, 512
P = 128
NKH = H // P  # 4 partition blocks
KERNEL = 64
OUT_VALID = H - KERNEL + 1  # 449
PT = (H - OUT_VALID) // 2  # 31 (left/top pad)
OLO, OHI = 28, 484  # computed output-row range (quad-aligned cover of valid)
NOUT = OHI - OLO  # 456 rows
JLO, JHI = PT, PT + OUT_VALID  # stored output-col range [31, 480)
NJ = JHI - JLO  # 449 cols
NP_OUT = NOUT // 4  # 114 output partitions
G = 4  # channels per grouped store (>=14KB descriptors)
NG_GROUPED = 15  # groups 0..14 cover channels 0..59
C_TAIL = NG_GROUPED * G  # 60: channels >= this store as split 2-ch groups
NPAIR = C // 2  # input channel pairs

# Matmul plan: contraction over standard 128-row blocks k, ONE
# instruction per k (4 LDWEIGHTS per block-pass instead of 7 — the
# per-column 64-row window spans at most 2 k-blocks, and PSUM's
# per-element has_written bit makes "accumulate onto a never-written
# element" a plain write, so a single instruction may mix first-writer
# and accumulating columns): (k, lo, hi, start, stop).
MM_PLAN_BLOCK = [
    (0, 0, 159, True, False),
    (1, 96, 287, False, False),
    (2, 224, 415, False, False),
    (3, 352, 512, False, True),
]
# Pass-1 plan: i-range clipped to the stored rows [OLO, OHI).
MM_PLAN_P1 = [
    (k, max(lo, OLO), min(hi, OHI), start, stop)
    for (k, lo, hi, start, stop) in MM_PLAN_BLOCK
]
# Pass-2 plan: j-range clipped to the valid cols [JLO, JHI).
MM_PLAN_P2 = [
    (k, max(lo, JLO), min(hi, JHI), start, stop)
    for (k, lo, hi, start, stop) in MM_PLAN_BLOCK
]


def make_band() -> np.ndarray:
    i = np.arange(H)
    ic = np.clip(i - PT, 0, OUT_VALID - 1)
    h = np.arange(H)
    band = (h[:, None] >= ic[None, :]) & (h[:, None] < ic[None, :] + KERNEL)
    band = (band.astype(np.float32) / KERNEL).astype(ml_dtypes.bfloat16)
    # pre-block to [p, k, i] = band[128k+p, i] so the DMA reads 4KB chunks
    return np.ascontiguousarray(band.reshape(NKH, P, W).transpose(1, 0, 2))


class CopyBalancer:
    """Greedy build-time assignment of PSUM->SBUF copies to the two
    engines with PSUM ports, weighted by their measured per-copy cost."""

    def __init__(self, nc):
        self.nc = nc
        self.load = {"scalar": 0.0, "vector": 0.0}

    def copy(self, dst, src, fd):
        cost = {"scalar": 250 + fd / 1.2, "vector": (120 + fd) / 0.96}
        eng = min(cost, key=lambda e: self.load[e] + cost[e])
        self.load[eng] += cost[eng]
        if eng == "scalar":
            self.nc.scalar.copy(dst, src)
        else:
            self.nc.vector.tensor_copy(dst, src)


def build_avgpool(tc, x_ap, band_ap, out_ap, out_tail_ap):
    import concourse.mybir as mybir

    nc = tc.nc
    f32 = mybir.dt.float32
    bf16 = mybir.dt.bfloat16
    cb = CopyBalancer(nc)

    with (
        tc.tile_pool(name="const", bufs=1) as const_pool,
        tc.tile_pool(name="xin", bufs=14) as xin_pool,
        tc.tile_pool(name="vt", bufs=3) as vt_pool,
        tc.tile_pool(name="oout", bufs=3) as out_pool,
        tc.tile_pool(name="otail", bufs=4) as tail_pool,
        tc.tile_pool(name="vtps", bufs=2, space="PSUM") as vt_psum,
        tc.tile_pool(name="ops", bufs=2, space="PSUM") as o_psum,
    ):
        # band pre-blocked on host: [p, k, i] = band[128*k + p, i];
        # one dma_start on the scalar engine lands on its first (early,
        # ~5.5 us ring-up) HWDGE queue, parallel to channel 0's load on
        # sync. (Splitting it or using the 2nd scalar ring starts ~4 us
        # later and delays the first matmul — measured.)
        band_t = const_pool.tile([P, NKH, H], bf16, tag="band")
        nc.scalar.dma_start(band_t[:], band_ap)

        # -------- input loads: per channel on sync, 4KB descriptors ----
        xtiles = {}

        def load_chan(c, splits=1):
            t = xin_pool.tile([P, NKH, W], bf16, tag="xb")
            if splits == 1:
                nc.sync.dma_start(t[:], x_ap[c])
            else:
                step = P // splits
                for s in range(splits):
                    sl = slice(step * s, step * (s + 1))
                    nc.sync.dma_start(t[sl], x_ap[c, sl])
            xtiles[c] = t

        # warmup: fine-split the first channels so their descriptors
        # land on several ~38-desc DMA engine windows in parallel (one
        # window moves only ~22.5 GB/s), then fill the prefetch window
        # (deep prefetch front-loads the read stream while stores are
        # scarce)
        load_chan(0, splits=4)
        load_chan(1, splits=2)
        for c in range(2, 14):
            load_chan(c)

        vtbs = {}

        def pass1(c):
            if c + 14 < C:
                load_chan(c + 14)
            xb = xtiles.pop(c)
            # vtb padded to 512 cols: pass-2 lhsT slices t::4 must be
            # 128-wide or FWL turns off (needs NumWeights==128) and
            # LDWEIGHTS runs 2x slow; pad cols feed only PSUM partitions
            # >= 114, which are never copied out
            vtb = vt_pool.tile([P, NKH, W], bf16, tag="vtb")
            vtbs[c] = vtb
            # 2-bank PSUM pair tiles: one copy per 2 mw-blocks halves
            # the per-copy fixed cost vs single banks
            for mp in range(2):
                vt_ps = vt_psum.tile([P, 2, W], f32, tag="vt")
                for sub in range(2):
                    mw = 2 * mp + sub
                    for k, lo, hi, start, stop in MM_PLAN_P1:
                        nc.tensor.matmul(
                            vt_ps[:, sub, lo - OLO : hi - OLO],
                            xb[:, k, P * mw : P * (mw + 1)],
                            band_t[:, k, lo:hi],
                            start=start,
                            stop=stop,
                        )
                cb.copy(
                    vtb[:, 2 * mp : 2 * mp + 2, :NOUT],
                    vt_ps[:, :, :NOUT],
                    2 * NOUT,
                )

        osbs = {}

        def pass2(c):
            g, ci = divmod(c, G)
            tail = c >= C_TAIL
            if tail:
                tg, tci = divmod(c - C_TAIL, 2)
                if tci == 0:
                    o_tl_new = tail_pool.tile([NP_OUT, 2, 4, NJ], bf16, tag="otl")
                    osbs[("t", tg)] = o_tl_new
                o_sb = osbs[("t", tg)]
                ci = tci
            else:
                if ci == 0:
                    o_sb_new = out_pool.tile([NP_OUT, G, 4, NJ], bf16, tag="osb")
                    osbs[g] = o_sb_new
                o_sb = osbs[g]
            vtb = vtbs.pop(c)
            for hf in range(2):
                # m-slices padded to 512 for PSUM bank alignment; only
                # j in [JLO, JHI) is computed — the trimmed columns are
                # edge replication the host reapplies
                o_ps = o_psum.tile([P, 2, W], f32, tag="o")
                for m in range(2):
                    t = 2 * hf + m
                    for k, lo, hi, start, stop in MM_PLAN_P2:
                        nc.tensor.matmul(
                            o_ps[:, m, lo - JLO : hi - JLO],
                            vtb[:, k, t : W : 4],
                            band_t[:, k, lo:hi],
                            start=start,
                            stop=stop,
                        )
                cb.copy(
                    o_sb[:, ci, 2 * hf : 2 * hf + 2, :],
                    o_ps[:NP_OUT, :, :NJ],
                    2 * NJ,
                )
            if tail and ci == 1:
                # Tail (c >= 56): 2-channel groups, each split into 6
                # partition-slices of 19 descriptors. A DMA engine
                # window is ~38 descriptors regardless of byte size, so
                # a whole group on one dma_start drains over only ~3
                # windows (~8us-24us); 19-desc slices each get their own
                # window (~135KB, ~6.2us) and the final group drains
                # right after its last copy. Queue choice avoids
                # serializing any one engine's ~0.6-0.9us-per-dispatch
                # stream: gpsimd for the first groups, sync once input
                # dispatching is idle, scalar only for the last group
                # (a blocked dma_start in scalar's stream would stall
                # its remaining copies). 7184B descriptors keep the
                # per-engine rate (3.6KB descs crawl).
                o_tl = osbs.pop(("t", tg))
                if tg == 0:
                    # (60,61): whole 2-ch group on gpsimd, drains in the
                    # shadow of (62,63)'s compute. (Partition-sliced
                    # fan-out on the SWDGE queue is poison: it blows up
                    # the end-of-kernel SWDGE DRAIN by tens of us.)
                    nc.gpsimd.dma_start(out_tail_ap[tg], o_tl[:])
                else:
                    # (62,63), the FINAL data: 8 partition-slices over
                    # the by-now-idle sync + scalar HWDGE rings -> ~8
                    # engine windows, ~100KB each, draining right after
                    # the last copy.
                    for s in range(8):
                        lo = (NP_OUT * s) // 8
                        hi = (NP_OUT * (s + 1)) // 8
                        eng = nc.sync if s % 2 == 0 else nc.scalar
                        eng.dma_start(out_tail_ap[tg, lo:hi], o_tl[lo:hi])
            elif not tail and ci == G - 1:
                # one >=14KB-per-partition store per 4-channel group,
                # dispatched from the otherwise-idle GpSimd (SWDGE)
                # queue (a HWDGE dispatch would block the sync/scalar
                # instruction stream on this group's copy dependencies,
                # starving input prefetch / later copies)
                nc.gpsimd.dma_start(out_ap[g], osbs.pop(g)[:])

        # software pipeline: PE runs pass1(c+1) while pass2(c) waits on
        # pass-1 copies
        pass1(0)
        for c in range(1, C):
            pass1(c)
            pass2(c - 1)
        pass2(C - 1)


def build_nc():
    import concourse.mybir as mybir
    import concourse.tile as tile
    from concourse import bacc

    # Bacc (not raw Bass): its compile() runs generate_event_semaphores,
    # which splits multi-semaphore waits — walrus codegen allows at most
    # one wait command per DMA instruction.
    nc = bacc.Bacc()
    x = nc.dram_tensor(
        "x", [C, P, NKH, W], mybir.dt.bfloat16, kind="ExternalInput"
    )
    band = nc.dram_tensor(
        "band", [P, NKH, H], mybir.dt.bfloat16, kind="ExternalInput"
    )
    out = nc.dram_tensor(
        "out",
        [NG_GROUPED, NP_OUT, G, 4, NJ],
        mybir.dt.bfloat16,
        kind="ExternalOutput",
    )
    out_tail = nc.dram_tensor(
        "out_tail",
        [(C - C_TAIL) // 2, NP_OUT, 2, 4, NJ],
        mybir.dt.bfloat16,
        kind="ExternalOutput",
    )
    with tile.TileContext(nc) as tc:
        build_avgpool(tc, x.ap(), band.ap(), out.ap(), out_tail.ap())
    nc.compile()
    return nc


def _ensure_axon_ntff_hook():
    """If tracing is requested (BASS_TRACE) under axon, run_bass_kernel_spmd
    imports antenv.axon_hooks, which some agent images lack. Install the
    real hook if possible, else a stub that degrades tracing gracefully."""
    import sys
    import types

    try:
        import antenv.axon_hooks  # noqa: F401

        return
    except Exception:
        pass
    try:
        import antenv
    except Exception:
        return
    mod = types.ModuleType("antenv.axon_hooks")
    mod._hook = None
    mod.set_axon_ntff_profile_hook = lambda h: setattr(mod, "_hook", h)
    mod.get_axon_ntff_profile_hook = lambda: mod._hook
    sys.modules["antenv.axon_hooks"] = mod
    antenv.axon_hooks = mod
    try:
        from trn_agent_boot.trn_boot import _ntff_profile_via_ctypes

        hook = _ntff_profile_via_ctypes("/opt/axon/libaxon_pjrt.so")
        if hook is not None:
            mod.set_axon_ntff_profile_hook(hook)
    except Exception:
        pass


def prep_inputs(x: np.ndarray):
    """Shard, cast, and pre-block the full input for the 8 cores.

    Device x layout: x_dev[c, p, k, w] = x[c, 128k+p, w]
    (4KB contiguous per partition per channel dma).
    """
    x = np.asarray(x, dtype=np.float32)
    assert x.shape == (8, C, H, W)
    xb = x.astype(ml_dtypes.bfloat16)
    band = make_band()
    in_maps = []
    for b in range(8):
        xd = np.ascontiguousarray(
            xb[b].reshape(C, NKH, P, W).transpose(0, 2, 1, 3)
        )
        in_maps.append({"x": xd, "band": band})
    return in_maps


def gather_output(results) -> np.ndarray:
    """Unshard, un-block, upcast, and re-apply the replicate padding.

    Device out layout (grouped, c < 60): out[g, p, ci, t, s] =
      out_full[4g+ci, OLO+4p+t, JLO+s]
    Device out layout (tail, c >= 60): out_tail[tg, p, tci, t, s] with
      c = 60 + 2*tg + tci, same row/col mapping.
    Stored rows [28, 484) include the replicated rows 28..30 / 480..483
    (the band clamp makes them exact copies of rows 31 / 479), so only
    rows outside [28, 484) and cols outside [31, 480) are reapplied.
    """
    full = np.empty((8, C, H, W), dtype=np.float32)
    for b, r in enumerate(results):
        og = np.asarray(r["out"])  # [15, 114, G, 4, 449] bf16
        ot = np.asarray(r["out_tail"])  # [2, 114, 2, 4, 449] bf16
        rows = np.empty((C, NOUT, NJ), dtype=np.float32)
        rows[:C_TAIL] = (
            og.transpose(0, 2, 1, 3, 4).reshape(C_TAIL, NOUT, NJ)
        )
        rows[C_TAIL:] = (
            ot.transpose(0, 2, 1, 3, 4).reshape(C - C_TAIL, NOUT, NJ)
        )
        full[b, :, OLO:OHI, JLO:JHI] = rows
        full[b, :, OLO:OHI, :JLO] = rows[:, :, :1]
        full[b, :, OLO:OHI, JHI:] = rows[:, :, -1:]
        full[b, :, :OLO, :] = full[b, :, OLO : OLO + 1, :]
        full[b, :, OHI:, :] = full[b, :, OHI - 1 : OHI, :]
    return full


def kernel(x) -> np.ndarray:
    _ensure_axon_ntff_hook()
    from concourse.bass_utils import run_bass_kernel_spmd

    nc = build_nc()
    in_maps = prep_inputs(x)
    res = run_bass_kernel_spmd(nc, in_maps, core_ids=list(range(8)))
    return gather_output(res.results)


# revision 24
# speedup vs baseline: 1.3231x; 1.0001x over previous
"""AvgPool2d(64x64, stride 1) with replicate-padding back to (512, 512),
as a distributed Bass kernel on 8 TRN2 NeuronCores.

Input : x (8, 64, 512, 512) float32
Output: (8, 64, 512, 512) float32

Strategy (pure data parallel): one batch element per core. Per core the
pooling is a separable 64-wide box filter; both directions run on the
TensorEngine as matmuls against a banded 0/1-matrix `band` [512, 512]
with band[h, i] = 1/64 iff clamp(i-31, 0, 448) <= h < clamp(...) + 64
(the clamp folds the replicate-padding, the 1/64 folds the averaging).

    V^T = (X^T @ band)        pass 1: vertical box mean, transposed
    O   = (V^T)^T @ band      pass 2: horizontal box mean, natural

Both passes keep the *data* tile in the stationary (lhsT) operand and
the band in the moving operand, which avoids every transpose.

v2 (trace-driven rework of the 216-222 us baseline):
- Tail: the baseline dispatched the last 4-channel group store (1.66MB)
  only after the final matmul -> ~26 us of unoverlapped drain (last
  matmul at 201.7 us, kernel end 228 us). Now channels 60..63 store
  per-channel, split into partition-quarter dma_starts on the sync
  queue (idle at kernel end), dispatched as soon as each channel's two
  PSUM->SBUF copies land.
- Start: the band was one dma_start -> one DMA engine (22.5 GB/s) and
  input c0 was a single dma_start; first matmul at 12.4 us. Now the
  band loads as 4 partition-slices on scalar (idle at t=0) and the
  first channels' loads are partition-split across many engines.
- Input loads ride sync as 2-channel pairs (8KB per-partition
  descriptors, 32 dma_starts instead of 64): halves the 565ns/dispatch
  sync-engine cost and the event-semaphore count.
- Pass-1 PSUM tiles are 2-bank pairs copied once ([128, 2, 456], one
  copy instead of two): ~11% less scalar/vector copy time. Copies stay
  greedily balanced between scalar and vector (the only PSUM-capable
  engines).
- Only output columns [31, 480) (the 449 valid ones) are computed in
  pass 2 / copied / stored; rows keep the quad-aligned [28, 484) cover.
  Store descriptors stay >= 14KB (4-channel groups) for full per-engine
  DMA rate (measured 23.9 GB/s at 14.6KB vs 177ns/4KB reads).
- Deep input prefetch (7 pairs = 14 channels in flight) front-loads the
  read stream while stores are scarce, and keeps the PE p-state hot
  (PE restarts at 1.2 GHz for 3 us after any idle; the trace showed
  30.5 us of throttle).
"""

import numpy as np
import ml_dtypes

C, H, W = 64, 512, 512
P = 128
NKH = H // P  # 4 partition blocks
KERNEL = 64
OUT_VALID = H - KERNEL + 1  # 449
PT = (H - OUT_VALID) // 2  # 31 (left/top pad)
OLO, OHI = 28, 484  # computed output-row range (quad-aligned cover of valid)
NOUT = OHI - OLO  # 456 rows
JLO, JHI = PT, PT + OUT_VALID  # stored output-col range [31, 480)
NJ = JHI - JLO  # 449 cols
NP_OUT = NOUT // 4  # 114 output partitions
G = 4  # channels per grouped store (>=14KB descriptors)
NG_GROUPED = 15  # groups 0..14 cover channels 0..59
C_TAIL = NG_GROUPED * G  # 60: channels >= this store as split 2-ch groups
NPAIR = C // 2  # input channel pairs

# Matmul plan: contraction over standard 128-row blocks k, ONE
# instruction per k (4 LDWEIGHTS per block-pass instead of 7 — the
# per-column 64-row window spans at most 2 k-blocks, and PSUM's
# per-element has_written bit makes "accumulate onto a never-written
# element" a plain write, so a single instruction may mix first-writer
# and accumulating columns): (k, lo, hi, start, stop).
MM_PLAN_BLOCK = [
    (0, 0, 159, True, False),
    (1, 96, 287, False, False),
    (2, 224, 415, False, False),
    (3, 352, 512, False, True),
]
# Pass-1 plan: i-range clipped to the stored rows [OLO, OHI).
MM_PLAN_P1 = [
    (k, max(lo, OLO), min(hi, OHI), start, stop)
    for (k, lo, hi, start, stop) in MM_PLAN_BLOCK
]
# Pass-2 plan: j-range clipped to the valid cols [JLO, JHI).
MM_PLAN_P2 = [
    (k, max(lo, JLO), min(hi, JHI), start, stop)
    for (k, lo, hi, start, stop) in MM_PLAN_BLOCK
]


def make_band() -> np.ndarray:
    i = np.arange(H)
    ic = np.clip(i - PT, 0, OUT_VALID - 1)
    h = np.arange(H)
    band = (h[:, None] >= ic[None, :]) & (h[:, None] < ic[None, :] + KERNEL)
    band = (band.astype(np.float32) / KERNEL).astype(ml_dtypes.bfloat16)
    # pre-block to [p, k, i] = band[128k+p, i] so the DMA reads 4KB chunks
    return np.ascontiguousarray(band.reshape(NKH, P, W).transpose(1, 0, 2))


class CopyBalancer:
    """Greedy build-time assignment of PSUM->SBUF copies to the two
    engines with PSUM ports, weighted by their measured per-copy cost."""

    def __init__(self, nc):
        self.nc = nc
        self.load = {"scalar": 0.0, "vector": 0.0}

    def copy(self, dst, src, fd):
        cost = {"scalar": 250 + fd / 1.2, "vector": (120 + fd) / 0.96}
        eng = min(cost, key=lambda e: self.load[e] + cost[e])
        self.load[eng] += cost[eng]
        if eng == "scalar":
            self.nc.scalar.copy(dst, src)
        else:
            self.nc.vector.tensor_copy(dst, src)


def build_avgpool(tc, x_ap, band_ap, out_ap, out_tail_ap):
    import concourse.mybir as mybir

    nc = tc.nc
    f32 = mybir.dt.float32
    bf16 = mybir.dt.bfloat16
    cb = CopyBalancer(nc)

    with (
        tc.tile_pool(name="const", bufs=1) as const_pool,
        tc.tile_pool(name="xin", bufs=14) as xin_pool,
        tc.tile_pool(name="vt", bufs=3) as vt_pool,
        tc.tile_pool(name="oout", bufs=3) as out_pool,
        tc.tile_pool(name="otail", bufs=4) as tail_pool,
        tc.tile_pool(name="vtps", bufs=2, space="PSUM") as vt_psum,
        tc.tile_pool(name="ops", bufs=2, space="PSUM") as o_psum,
    ):
        # -------- input loads: per channel, 4KB descriptors --------
        xtiles = {}

        def load_chan(c, eng=None):
            t = xin_pool.tile([P, NKH, W], bf16, tag="xb")
            (eng or nc.sync).dma_start(t[:], x_ap[c])
            xtiles[c] = t

        # Channel 0 and the band ride the SCALAR engine's first HWDGE
        # ring: the framework's act-table load warms it by ~2.6 us,
        # while the sync ring's first packet only moves at ~8.7 us.
        # This pulls the first matmul ~4 us earlier.
        load_chan(0, eng=nc.scalar)
        # band pre-blocked on host: [p, k, i] = band[128*k + p, i]
        band_t = const_pool.tile([P, NKH, H], bf16, tag="band")
        nc.scalar.dma_start(band_t[:], band_ap)
        # warmup: fill the prefetch window on sync (deep prefetch
        # front-loads the read stream while stores are scarce)
        for c in range(1, 14):
            load_chan(c)

        vtbs = {}

        def pass1(c):
            if c + 14 < C:
                load_chan(c + 14)
            xb = xtiles.pop(c)
            # vtb padded to 512 cols: pass-2 lhsT slices t::4 must be
            # 128-wide or FWL turns off (needs NumWeights==128) and
            # LDWEIGHTS runs 2x slow; pad cols feed only PSUM partitions
            # >= 114, which are never copied out
            vtb = vt_pool.tile([P, NKH, W], bf16, tag="vtb")
            vtbs[c] = vtb
            # 2-bank PSUM pair tiles: one copy per 2 mw-blocks halves
            # the per-copy fixed cost vs single banks
            for mp in range(2):
                vt_ps = vt_psum.tile([P, 2, W], f32, tag="vt")
                for sub in range(2):
                    mw = 2 * mp + sub
                    for k, lo, hi, start, stop in MM_PLAN_P1:
                        nc.tensor.matmul(
                            vt_ps[:, sub, lo - OLO : hi - OLO],
                            xb[:, k, P * mw : P * (mw + 1)],
                            band_t[:, k, lo:hi],
                            start=start,
                            stop=stop,
                        )
                cb.copy(
                    vtb[:, 2 * mp : 2 * mp + 2, :NOUT],
                    vt_ps[:, :, :NOUT],
                    2 * NOUT,
                )

        osbs = {}

        def pass2(c):
            g, ci = divmod(c, G)
            tail = c >= C_TAIL
            if tail:
                tg, tci = divmod(c - C_TAIL, 2)
                if tci == 0:
                    o_tl_new = tail_pool.tile([NP_OUT, 2, 4, NJ], bf16, tag="otl")
                    osbs[("t", tg)] = o_tl_new
                o_sb = osbs[("t", tg)]
                ci = tci
            else:
                if ci == 0:
                    o_sb_new = out_pool.tile([NP_OUT, G, 4, NJ], bf16, tag="osb")
                    osbs[g] = o_sb_new
                o_sb = osbs[g]
            vtb = vtbs.pop(c)
            for hf in range(2):
                # m-slices padded to 512 for PSUM bank alignment; only
                # j in [JLO, JHI) is computed — the trimmed columns are
                # edge replication the host reapplies
                o_ps = o_psum.tile([P, 2, W], f32, tag="o")
                for m in range(2):
                    t = 2 * hf + m
                    for k, lo, hi, start, stop in MM_PLAN_P2:
                        nc.tensor.matmul(
                            o_ps[:, m, lo - JLO : hi - JLO],
                            vtb[:, k, t : W : 4],
                            band_t[:, k, lo:hi],
                            start=start,
                            stop=stop,
                        )
                cb.copy(
                    o_sb[:, ci, 2 * hf : 2 * hf + 2, :],
                    o_ps[:NP_OUT, :, :NJ],
                    2 * NJ,
                )
            if tail and ci == 1:
                # Tail (c >= 56): 2-channel groups, each split into 6
                # partition-slices of 19 descriptors. A DMA engine
                # window is ~38 descriptors regardless of byte size, so
                # a whole group on one dma_start drains over only ~3
                # windows (~8us-24us); 19-desc slices each get their own
                # window (~135KB, ~6.2us) and the final group drains
                # right after its last copy. Queue choice avoids
                # serializing any one engine's ~0.6-0.9us-per-dispatch
                # stream: gpsimd for the first groups, sync once input
                # dispatching is idle, scalar only for the last group
                # (a blocked dma_start in scalar's stream would stall
                # its remaining copies). 7184B descriptors keep the
                # per-engine rate (3.6KB descs crawl).
                o_tl = osbs.pop(("t", tg))
                if tg == 0:
                    # (60,61): whole 2-ch group on gpsimd, drains in the
                    # shadow of (62,63)'s compute. (Partition-sliced
                    # fan-out on the SWDGE queue is poison: it blows up
                    # the end-of-kernel SWDGE DRAIN by tens of us.)
                    nc.gpsimd.dma_start(out_tail_ap[tg], o_tl[:])
                else:
                    # (62,63), the FINAL data: 8 partition-slices over
                    # the by-now-idle sync + scalar HWDGE rings -> ~8
                    # engine windows, ~100KB each, draining right after
                    # the last copy.
                    for s in range(8):
                        lo = (NP_OUT * s) // 8
                        hi = (NP_OUT * (s + 1)) // 8
                        eng = nc.sync if s % 2 == 0 else nc.scalar
                        eng.dma_start(out_tail_ap[tg, lo:hi], o_tl[lo:hi])
            elif not tail and ci == 1:
                # bulk stores as two half-group dma_starts (7184B
                # descriptors) from the otherwise-idle GpSimd (SWDGE)
                # queue (a HWDGE dispatch would block the sync/scalar
                # instruction stream on this group's copy dependencies,
                # starving input prefetch / later copies). Halved
                # dispatches halve the time a store holds a DMA engine
                # window (~38 descs regardless of size): whole 14.4KB
                # groups hog 3 engines for ~24 us each, head-of-line
                # blocking input-read chunks and stalling the PE at the
                # group cadence.
                nc.gpsimd.dma_start(out_ap[g, :, 0:2], o_sb[:, 0:2])
            elif not tail and ci == G - 1:
                nc.gpsimd.dma_start(out_ap[g, :, 2:4], osbs.pop(g)[:, 2:4])

        # software pipeline: PE runs pass1(c+1) while pass2(c) waits on
        # pass-1 copies
        pass1(0)
        for c in range(1, C):
            pass1(c)
            pass2(c - 1)
        pass2(C - 1)


def build_nc():
    import concourse.mybir as mybir
    import concourse.tile as tile
    from concourse import bacc

    # Bacc (not raw Bass): its compile() runs generate_event_semaphores,
    # which splits multi-semaphore waits — walrus codegen allows at most
    # one wait command per DMA instruction.
    nc = bacc.Bacc()
    x = nc.dram_tensor(
        "x", [C, P, NKH, W], mybir.dt.bfloat16, kind="ExternalInput"
    )
    band = nc.dram_tensor(
        "band", [P, NKH, H], mybir.dt.bfloat16, kind="ExternalInput"
    )
    out = nc.dram_tensor(
        "out",
        [NG_GROUPED, NP_OUT, G, 4, NJ],
        mybir.dt.bfloat16,
        kind="ExternalOutput",
    )
    out_tail = nc.dram_tensor(
        "out_tail",
        [(C - C_TAIL) // 2, NP_OUT, 2, 4, NJ],
        mybir.dt.bfloat16,
        kind="ExternalOutput",
    )
    with tile.TileContext(nc) as tc:
        build_avgpool(tc, x.ap(), band.ap(), out.ap(), out_tail.ap())
    nc.compile()
    return nc


def _ensure_axon_ntff_hook():
    """If tracing is requested (BASS_TRACE) under axon, run_bass_kernel_spmd
    imports antenv.axon_hooks, which some agent images lack. Install the
    real hook if possible, else a stub that degrades tracing gracefully."""
    import sys
    import types

    try:
        import antenv.axon_hooks  # noqa: F401

        return
    except Exception:
        pass
    try:
        import antenv
    except Exception:
        return
    mod = types.ModuleType("antenv.axon_hooks")
    mod._hook = None
    mod.set_axon_ntff_profile_hook = lambda h: setattr(mod, "_hook", h)
    mod.get_axon_ntff_profile_hook = lambda: mod._hook
    sys.modules["antenv.axon_hooks"] = mod
    antenv.axon_hooks = mod
    try:
        from trn_agent_boot.trn_boot import _ntff_profile_via_ctypes

        hook = _ntff_profile_via_ctypes("/opt/axon/libaxon_pjrt.so")
        if hook is not None:
            mod.set_axon_ntff_profile_hook(hook)
    except Exception:
        pass


def prep_inputs(x: np.ndarray):
    """Shard, cast, and pre-block the full input for the 8 cores.

    Device x layout: x_dev[c, p, k, w] = x[c, 128k+p, w]
    (4KB contiguous per partition per channel dma).
    """
    x = np.asarray(x, dtype=np.float32)
    assert x.shape == (8, C, H, W)
    xb = x.astype(ml_dtypes.bfloat16)
    band = make_band()
    in_maps = []
    for b in range(8):
        xd = np.ascontiguousarray(
            xb[b].reshape(C, NKH, P, W).transpose(0, 2, 1, 3)
        )
        in_maps.append({"x": xd, "band": band})
    return in_maps


def gather_output(results) -> np.ndarray:
    """Unshard, un-block, upcast, and re-apply the replicate padding.

    Device out layout (grouped, c < 60): out[g, p, ci, t, s] =
      out_full[4g+ci, OLO+4p+t, JLO+s]
    Device out layout (tail, c >= 60): out_tail[tg, p, tci, t, s] with
      c = 60 + 2*tg + tci, same row/col mapping.
    Stored rows [28, 484) include the replicated rows 28..30 / 480..483
    (the band clamp makes them exact copies of rows 31 / 479), so only
    rows outside [28, 484) and cols outside [31, 480) are reapplied.
    """
    full = np.empty((8, C, H, W), dtype=np.float32)
    for b, r in enumerate(results):
        og = np.asarray(r["out"])  # [15, 114, G, 4, 449] bf16
        ot = np.asarray(r["out_tail"])  # [2, 114, 2, 4, 449] bf16
        rows = np.empty((C, NOUT, NJ), dtype=np.float32)
        rows[:C_TAIL] = (
            og.transpose(0, 2, 1, 3, 4).reshape(C_TAIL, NOUT, NJ)
        )
        rows[C_TAIL:] = (
            ot.transpose(0, 2, 1, 3, 4).reshape(C - C_TAIL, NOUT, NJ)
        )
        full[b, :, OLO:OHI, JLO:JHI] = rows
        full[b, :, OLO:OHI, :JLO] = rows[:, :, :1]
        full[b, :, OLO:OHI, JHI:] = rows[:, :, -1:]
        full[b, :, :OLO, :] = full[b, :, OLO : OLO + 1, :]
        full[b, :, OHI:, :] = full[b, :, OHI - 1 : OHI, :]
    return full


def kernel(x) -> np.ndarray:
    _ensure_axon_ntff_hook()
    from concourse.bass_utils import run_bass_kernel_spmd

    nc = build_nc()
    in_maps = prep_inputs(x)
    res = run_bass_kernel_spmd(nc, in_maps, core_ids=list(range(8)))
    return gather_output(res.results)


# revision 25
# speedup vs baseline: 1.3426x; 1.0148x over previous
"""AvgPool2d(64x64, stride 1) with replicate-padding back to (512, 512),
as a distributed Bass kernel on 8 TRN2 NeuronCores.

Input : x (8, 64, 512, 512) float32
Output: (8, 64, 512, 512) float32

Strategy (pure data parallel): one batch element per core. Per core the
pooling is a separable 64-wide box filter; both directions run on the
TensorEngine as matmuls against a banded 0/1-matrix `band` [512, 512]
with band[h, i] = 1/64 iff clamp(i-31, 0, 448) <= h < clamp(...) + 64
(the clamp folds the replicate-padding, the 1/64 folds the averaging).

    V^T = (X^T @ band)        pass 1: vertical box mean, transposed
    O   = (V^T)^T @ band      pass 2: horizontal box mean, natural

Both passes keep the *data* tile in the stationary (lhsT) operand and
the band in the moving operand, which avoids every transpose.

v2 (trace-driven rework of the 216-222 us baseline):
- Tail: the baseline dispatched the last 4-channel group store (1.66MB)
  only after the final matmul -> ~26 us of unoverlapped drain (last
  matmul at 201.7 us, kernel end 228 us). Now channels 60..63 store
  per-channel, split into partition-quarter dma_starts on the sync
  queue (idle at kernel end), dispatched as soon as each channel's two
  PSUM->SBUF copies land.
- Start: the band was one dma_start -> one DMA engine (22.5 GB/s) and
  input c0 was a single dma_start; first matmul at 12.4 us. Now the
  band loads as 4 partition-slices on scalar (idle at t=0) and the
  first channels' loads are partition-split across many engines.
- Input loads ride sync as 2-channel pairs (8KB per-partition
  descriptors, 32 dma_starts instead of 64): halves the 565ns/dispatch
  sync-engine cost and the event-semaphore count.
- Pass-1 PSUM tiles are 2-bank pairs copied once ([128, 2, 456], one
  copy instead of two): ~11% less scalar/vector copy time. Copies stay
  greedily balanced between scalar and vector (the only PSUM-capable
  engines).
- Only output columns [31, 480) (the 449 valid ones) are computed in
  pass 2 / copied / stored; rows keep the quad-aligned [28, 484) cover.
  Store descriptors stay >= 14KB (4-channel groups) for full per-engine
  DMA rate (measured 23.9 GB/s at 14.6KB vs 177ns/4KB reads).
- Deep input prefetch (7 pairs = 14 channels in flight) front-loads the
  read stream while stores are scarce, and keeps the PE p-state hot
  (PE restarts at 1.2 GHz for 3 us after any idle; the trace showed
  30.5 us of throttle).
"""

import numpy as np
import ml_dtypes

C, H, W = 64, 512, 512
P = 128
NKH = H // P  # 4 partition blocks
KERNEL = 64
OUT_VALID = H - KERNEL + 1  # 449
PT = (H - OUT_VALID) // 2  # 31 (left/top pad)
OLO, OHI = 28, 484  # computed output-row range (quad-aligned cover of valid)
NOUT = OHI - OLO  # 456 rows
JLO, JHI = PT, PT + OUT_VALID  # stored output-col range [31, 480)
NJ = JHI - JLO  # 449 cols
NP_OUT = NOUT // 4  # 114 output partitions
G = 4  # channels per grouped store (>=14KB descriptors)
NG_GROUPED = 15  # groups 0..14 cover channels 0..59
C_TAIL = NG_GROUPED * G  # 60: channels >= this store as split 2-ch groups
NPAIR = C // 2  # input channel pairs

# Matmul plan: contraction over standard 128-row blocks k, ONE
# instruction per k (4 LDWEIGHTS per block-pass instead of 7 — the
# per-column 64-row window spans at most 2 k-blocks, and PSUM's
# per-element has_written bit makes "accumulate onto a never-written
# element" a plain write, so a single instruction may mix first-writer
# and accumulating columns): (k, lo, hi, start, stop).
MM_PLAN_BLOCK = [
    (0, 0, 159, True, False),
    (1, 96, 287, False, False),
    (2, 224, 415, False, False),
    (3, 352, 512, False, True),
]
# Pass-1 plan: i-range clipped to the stored rows [OLO, OHI).
MM_PLAN_P1 = [
    (k, max(lo, OLO), min(hi, OHI), start, stop)
    for (k, lo, hi, start, stop) in MM_PLAN_BLOCK
]
# Pass-2 plan: j-range clipped to the valid cols [JLO, JHI).
MM_PLAN_P2 = [
    (k, max(lo, JLO), min(hi, JHI), start, stop)
    for (k, lo, hi, start, stop) in MM_PLAN_BLOCK
]


def make_band() -> np.ndarray:
    i = np.arange(H)
    ic = np.clip(i - PT, 0, OUT_VALID - 1)
    h = np.arange(H)
    band = (h[:, None] >= ic[None, :]) & (h[:, None] < ic[None, :] + KERNEL)
    band = (band.astype(np.float32) / KERNEL).astype(ml_dtypes.bfloat16)
    # pre-block to [p, k, i] = band[128k+p, i] so the DMA reads 4KB chunks
    return np.ascontiguousarray(band.reshape(NKH, P, W).transpose(1, 0, 2))


class CopyBalancer:
    """Greedy build-time assignment of PSUM->SBUF copies to the two
    engines with PSUM ports, weighted by their measured per-copy cost."""

    def __init__(self, nc):
        self.nc = nc
        self.load = {"scalar": 0.0, "vector": 0.0}

    def copy(self, dst, src, fd):
        cost = {"scalar": 250 + fd / 1.2, "vector": (120 + fd) / 0.96}
        eng = min(cost, key=lambda e: self.load[e] + cost[e])
        self.load[eng] += cost[eng]
        if eng == "scalar":
            self.nc.scalar.copy(dst, src)
        else:
            self.nc.vector.tensor_copy(dst, src)


def build_avgpool(tc, x_ap, band_ap, out_ap, out_tail_ap):
    import concourse.mybir as mybir

    nc = tc.nc
    f32 = mybir.dt.float32
    bf16 = mybir.dt.bfloat16
    cb = CopyBalancer(nc)

    with (
        tc.tile_pool(name="const", bufs=1) as const_pool,
        tc.tile_pool(name="xin", bufs=20) as xin_pool,
        tc.tile_pool(name="vt", bufs=3) as vt_pool,
        tc.tile_pool(name="oout", bufs=4) as out_pool,
        tc.tile_pool(name="otail", bufs=4) as tail_pool,
        tc.tile_pool(name="vtps", bufs=2, space="PSUM") as vt_psum,
        tc.tile_pool(name="ops", bufs=2, space="PSUM") as o_psum,
    ):
        # -------- input loads: per channel, 4KB descriptors --------
        xtiles = {}

        def load_chan(c, eng=None):
            t = xin_pool.tile([P, NKH, W], bf16, tag="xb")
            (eng or nc.sync).dma_start(t[:], x_ap[c])
            xtiles[c] = t

        # Channel 0 and the band ride the SCALAR engine's first HWDGE
        # ring: the framework's act-table load warms it by ~2.6 us,
        # while the sync ring's first packet only moves at ~8.7 us.
        # This pulls the first matmul ~4 us earlier.
        load_chan(0, eng=nc.scalar)
        # band pre-blocked on host: [p, k, i] = band[128*k + p, i]
        band_t = const_pool.tile([P, NKH, H], bf16, tag="band")
        nc.scalar.dma_start(band_t[:], band_ap)
        # warmup: fill the prefetch window on sync (deep prefetch
        # front-loads the read stream while stores are scarce)
        for c in range(1, 20):
            load_chan(c)

        vtbs = {}

        def pass1(c):
            if c + 20 < C:
                load_chan(c + 20)
            xb = xtiles.pop(c)
            # vtb padded to 512 cols: pass-2 lhsT slices t::4 must be
            # 128-wide or FWL turns off (needs NumWeights==128) and
            # LDWEIGHTS runs 2x slow; pad cols feed only PSUM partitions
            # >= 114, which are never copied out
            vtb = vt_pool.tile([P, NKH, W], bf16, tag="vtb")
            vtbs[c] = vtb
            # 2-bank PSUM pair tiles: one copy per 2 mw-blocks halves
            # the per-copy fixed cost vs single banks
            for mp in range(2):
                vt_ps = vt_psum.tile([P, 2, W], f32, tag="vt")
                for sub in range(2):
                    mw = 2 * mp + sub
                    for k, lo, hi, start, stop in MM_PLAN_P1:
                        nc.tensor.matmul(
                            vt_ps[:, sub, lo - OLO : hi - OLO],
                            xb[:, k, P * mw : P * (mw + 1)],
                            band_t[:, k, lo:hi],
                            start=start,
                            stop=stop,
                        )
                cb.copy(
                    vtb[:, 2 * mp : 2 * mp + 2, :NOUT],
                    vt_ps[:, :, :NOUT],
                    2 * NOUT,
                )

        osbs = {}

        def pass2(c):
            g, ci = divmod(c, G)
            tail = c >= C_TAIL
            if tail:
                tg, tci = divmod(c - C_TAIL, 2)
                if tci == 0:
                    o_tl_new = tail_pool.tile([NP_OUT, 2, 4, NJ], bf16, tag="otl")
                    osbs[("t", tg)] = o_tl_new
                o_sb = osbs[("t", tg)]
                ci = tci
            else:
                if ci == 0:
                    o_sb_new = out_pool.tile([NP_OUT, G, 4, NJ], bf16, tag="osb")
                    osbs[g] = o_sb_new
                o_sb = osbs[g]
            vtb = vtbs.pop(c)
            for hf in range(2):
                # m-slices padded to 512 for PSUM bank alignment; only
                # j in [JLO, JHI) is computed — the trimmed columns are
                # edge replication the host reapplies
                o_ps = o_psum.tile([P, 2, W], f32, tag="o")
                for m in range(2):
                    t = 2 * hf + m
                    for k, lo, hi, start, stop in MM_PLAN_P2:
                        nc.tensor.matmul(
                            o_ps[:, m, lo - JLO : hi - JLO],
                            vtb[:, k, t : W : 4],
                            band_t[:, k, lo:hi],
                            start=start,
                            stop=stop,
                        )
                cb.copy(
                    o_sb[:, ci, 2 * hf : 2 * hf + 2, :],
                    o_ps[:NP_OUT, :, :NJ],
                    2 * NJ,
                )
            if tail and ci == 1:
                # Tail (c >= 56): 2-channel groups, each split into 6
                # partition-slices of 19 descriptors. A DMA engine
                # window is ~38 descriptors regardless of byte size, so
                # a whole group on one dma_start drains over only ~3
                # windows (~8us-24us); 19-desc slices each get their own
                # window (~135KB, ~6.2us) and the final group drains
                # right after its last copy. Queue choice avoids
                # serializing any one engine's ~0.6-0.9us-per-dispatch
                # stream: gpsimd for the first groups, sync once input
                # dispatching is idle, scalar only for the last group
                # (a blocked dma_start in scalar's stream would stall
                # its remaining copies). 7184B descriptors keep the
                # per-engine rate (3.6KB descs crawl).
                o_tl = osbs.pop(("t", tg))
                if tg == 0:
                    # (60,61): whole 2-ch group on gpsimd, drains in the
                    # shadow of (62,63)'s compute. (Partition-sliced
                    # fan-out on the SWDGE queue is poison: it blows up
                    # the end-of-kernel SWDGE DRAIN by tens of us.)
                    nc.gpsimd.dma_start(out_tail_ap[tg], o_tl[:])
                else:
                    # (62,63), the FINAL data: 8 partition-slices over
                    # the by-now-idle sync + scalar HWDGE rings -> ~8
                    # engine windows, ~100KB each, draining right after
                    # the last copy.
                    for s in range(8):
                        lo = (NP_OUT * s) // 8
                        hi = (NP_OUT * (s + 1)) // 8
                        eng = nc.sync if s % 2 == 0 else nc.scalar
                        eng.dma_start(out_tail_ap[tg, lo:hi], o_tl[lo:hi])
            elif not tail and ci == 1:
                # bulk stores as two half-group dma_starts (7184B
                # descriptors) from the otherwise-idle GpSimd (SWDGE)
                # queue (a HWDGE dispatch would block the sync/scalar
                # instruction stream on this group's copy dependencies,
                # starving input prefetch / later copies). Halved
                # dispatches halve the time a store holds a DMA engine
                # window (~38 descs regardless of size): whole 14.4KB
                # groups hog 3 engines for ~24 us each, head-of-line
                # blocking input-read chunks and stalling the PE at the
                # group cadence.
                nc.gpsimd.dma_start(out_ap[g, :, 0:2], o_sb[:, 0:2])
            elif not tail and ci == G - 1:
                nc.gpsimd.dma_start(out_ap[g, :, 2:4], osbs.pop(g)[:, 2:4])

        # software pipeline: PE runs pass1(c+1) while pass2(c) waits on
        # pass-1 copies
        pass1(0)
        for c in range(1, C):
            pass1(c)
            pass2(c - 1)
        pass2(C - 1)


def build_nc():
    import concourse.mybir as mybir
    import concourse.tile as tile
    from concourse import bacc

    # Bacc (not raw Bass): its compile() runs generate_event_semaphores,
    # which splits multi-semaphore waits — walrus codegen allows at most
    # one wait command per DMA instruction.
    nc = bacc.Bacc()
    x = nc.dram_tensor(
        "x", [C, P, NKH, W], mybir.dt.bfloat16, kind="ExternalInput"
    )
    band = nc.dram_tensor(
        "band", [P, NKH, H], mybir.dt.bfloat16, kind="ExternalInput"
    )
    out = nc.dram_tensor(
        "out",
        [NG_GROUPED, NP_OUT, G, 4, NJ],
        mybir.dt.bfloat16,
        kind="ExternalOutput",
    )
    out_tail = nc.dram_tensor(
        "out_tail",
        [(C - C_TAIL) // 2, NP_OUT, 2, 4, NJ],
        mybir.dt.bfloat16,
        kind="ExternalOutput",
    )
    with tile.TileContext(nc) as tc:
        build_avgpool(tc, x.ap(), band.ap(), out.ap(), out_tail.ap())
    nc.compile()
    return nc


def _ensure_axon_ntff_hook():
    """If tracing is requested (BASS_TRACE) under axon, run_bass_kernel_spmd
    imports antenv.axon_hooks, which some agent images lack. Install the
    real hook if possible, else a stub that degrades tracing gracefully."""
    import sys
    import types

    try:
        import antenv.axon_hooks  # noqa: F401

        return
    except Exception:
        pass
    try:
        import antenv
    except Exception:
        return
    mod = types.ModuleType("antenv.axon_hooks")
    mod._hook = None
    mod.set_axon_ntff_profile_hook = lambda h: setattr(mod, "_hook", h)
    mod.get_axon_ntff_profile_hook = lambda: mod._hook
    sys.modules["antenv.axon_hooks"] = mod
    antenv.axon_hooks = mod
    try:
        from trn_agent_boot.trn_boot import _ntff_profile_via_ctypes

        hook = _ntff_profile_via_ctypes("/opt/axon/libaxon_pjrt.so")
        if hook is not None:
            mod.set_axon_ntff_profile_hook(hook)
    except Exception:
        pass


def prep_inputs(x: np.ndarray):
    """Shard, cast, and pre-block the full input for the 8 cores.

    Device x layout: x_dev[c, p, k, w] = x[c, 128k+p, w]
    (4KB contiguous per partition per channel dma).
    """
    x = np.asarray(x, dtype=np.float32)
    assert x.shape == (8, C, H, W)
    xb = x.astype(ml_dtypes.bfloat16)
    band = make_band()
    in_maps = []
    for b in range(8):
        xd = np.ascontiguousarray(
            xb[b].reshape(C, NKH, P, W).transpose(0, 2, 1, 3)
        )
        in_maps.append({"x": xd, "band": band})
    return in_maps


def gather_output(results) -> np.ndarray:
    """Unshard, un-block, upcast, and re-apply the replicate padding.

    Device out layout (grouped, c < 60): out[g, p, ci, t, s] =
      out_full[4g+ci, OLO+4p+t, JLO+s]
    Device out layout (tail, c >= 60): out_tail[tg, p, tci, t, s] with
      c = 60 + 2*tg + tci, same row/col mapping.
    Stored rows [28, 484) include the replicated rows 28..30 / 480..483
    (the band clamp makes them exact copies of rows 31 / 479), so only
    rows outside [28, 484) and cols outside [31, 480) are reapplied.
    """
    full = np.empty((8, C, H, W), dtype=np.float32)
    for b, r in enumerate(results):
        og = np.asarray(r["out"])  # [15, 114, G, 4, 449] bf16
        ot = np.asarray(r["out_tail"])  # [2, 114, 2, 4, 449] bf16
        rows = np.empty((C, NOUT, NJ), dtype=np.float32)
        rows[:C_TAIL] = (
            og.transpose(0, 2, 1, 3, 4).reshape(C_TAIL, NOUT, NJ)
        )
        rows[C_TAIL:] = (
            ot.transpose(0, 2, 1, 3, 4).reshape(C - C_TAIL, NOUT, NJ)
        )
        full[b, :, OLO:OHI, JLO:JHI] = rows
        full[b, :, OLO:OHI, :JLO] = rows[:, :, :1]
        full[b, :, OLO:OHI, JHI:] = rows[:, :, -1:]
        full[b, :, :OLO, :] = full[b, :, OLO : OLO + 1, :]
        full[b, :, OHI:, :] = full[b, :, OHI - 1 : OHI, :]
    return full


def kernel(x) -> np.ndarray:
    _ensure_axon_ntff_hook()
    from concourse.bass_utils import run_bass_kernel_spmd

    nc = build_nc()
    in_maps = prep_inputs(x)
    res = run_bass_kernel_spmd(nc, in_maps, core_ids=list(range(8)))
    return gather_output(res.results)
